# revision 12
# baseline (speedup 1.0000x reference)
"""Trainium2 Bass kernel for the buggy CrossAttention module.

Semantics replicated exactly from the reference (including its bugs):
  q = x @ q_w.T + q_b                  (k is computed-but-unused -> skipped)
  v = cross @ v_w.T + v_b
  scores = q_h . v_h / 8   (per head, "k" replaced by v per source bug)
  attn = softmax(scores)
  qkv = attn @ v_h
  qkv "reshaped" (B,H,L,DH)->(B,L,E) WITHOUT transposing back (source bug):
     out_row t' = h*(L/16) + t//16 contains tokens 16g..16g+15 concatenated
  out = qkv_reshaped @ o_w.T + o_b

Distribution: batch (16) sharded over 8 cores, 2 batches/core, no collectives.

All matmuls run in bf16 (inputs rounded to bf16, fp32 PSUM accumulation).
Layout strategy:
  - x^T, w^T produced via fp32->bf16 cast-DMA (SWDGE) + XBAR transpose-DMA
  - q^T [f, t] tiles from q-proj directly feed per-head attention
  - scores computed transposed [c, t] with v^T_h stationary
  - softmax denominator + partition-broadcast via ones-matmul on the PE;
    reciprocal on DVE, normalize multiply on GPSIMD (engine balance)
  - qkv computed per r = t%16 with strided moving operand, which emits the
    buggy-reshape layout for free
  - o-proj consumes qkv^T tiles as stationaries, bias via ones-row matmul
"""

import sys

for _p in ("/opt/trn_rl_repo",):
    if _p not in sys.path:
        sys.path.append(_p)

import numpy as np

import concourse.bass as bass
import concourse.mybir as mybir
import concourse.tile as tile
from concourse import bacc
from concourse.masks import make_identity

FP32 = mybir.dt.float32
BF16 = mybir.dt.bfloat16

B_TOTAL = 16
N_CORES = 8
B_CORE = B_TOTAL // N_CORES  # 2
L = 4096
E = 1024
LC = 77
EC = 768
H = 16
DH = 64

T_CHUNK = 512            # PSUM free-dim chunk (one fp32 bank)
FT = E // 128            # 8 f-tiles
ET = E // 128            # 8 e-tiles (contraction for q/o proj)
ECT = EC // 128          # 6 e-tiles (contraction for v proj)


def emit(tc, out_d, x_d, cross_d, qw_d, qb_d, vw_d, vb_d, ow_d, ob_d,
         b_core=B_CORE, l_tokens=L):
    nc = tc.nc
    t_block = min(4096, l_tokens)    # tokens per attention block
    g_block = t_block // 16          # g-window per block
    t_half = t_block // 2            # x-prep granularity
    n_blocks = l_tokens // t_block
    lt16 = l_tokens // 16
    ident = mybir.ActivationFunctionType.Identity
    expf = mybir.ActivationFunctionType.Exp

    # ---- internal DRAM (bf16 staging for transposes) ----
    x_bf = nc.dram_tensor("x_bf", [b_core, l_tokens, E], BF16, kind="Internal")
    qw_bf = nc.dram_tensor("qw_bf", [E, E], BF16, kind="Internal")
    ow_bf = nc.dram_tensor("ow_bf", [E, E], BF16, kind="Internal")
    vw_bf = nc.dram_tensor("vw_bf", [E, EC], BF16, kind="Internal")

    nc.gpsimd.dma_start(out=qw_bf[:, :], in_=qw_d[:, :])
    nc.gpsimd.dma_start(out=ow_bf[:, :], in_=ow_d[:, :])
    nc.gpsimd.dma_start(out=vw_bf[:, :], in_=vw_d[:, :])
    # x casts issued up-front: pure DMA, overlaps all later compute
    for b in range(b_core):
        for t0 in range(0, l_tokens, t_half):
            nc.gpsimd.dma_start(out=x_bf[b, t0:t0 + t_half, :],
                                in_=x_d[b, t0:t0 + t_half, :])

    import contextlib
    with contextlib.ExitStack() as ctx:
        consts = ctx.enter_context(tc.tile_pool(name="consts", bufs=1))
        identity = consts.tile([128, 128], BF16)
        make_identity(nc, identity)
        ones77 = consts.tile([LC, LC], BF16)
        nc.vector.memset(ones77, 1.0)
        ones1 = consts.tile([1, 128], BF16)
        nc.vector.memset(ones1, 1.0)
        ob_sb = consts.tile([1, E], BF16)
        nc.gpsimd.dma_start(out=ob_sb, in_=ob_d[:].rearrange("(a b) -> a b", a=1))
        qb_sb = consts.tile([128, FT], FP32)
        vb_sb = consts.tile([128, FT], FP32)
        for j in range(FT):
            nc.sync.dma_start(
                out=qb_sb[:, j:j + 1],
                in_=qb_d[128 * j:128 * (j + 1)].rearrange("(a b) -> a b", b=1))
            nc.sync.dma_start(
                out=vb_sb[:, j:j + 1],
                in_=vb_d[128 * j:128 * (j + 1)].rearrange("(a b) -> a b", b=1))

        # ---- transposed weights in SBUF (bf16) ----
        wpool = ctx.enter_context(tc.tile_pool(name="wT", bufs=1))
        qwT = []
        owT = []
        vwT = []
        for e in range(ET):
            t_q = wpool.tile([128, E], BF16, name=f"qwT{e}")
            nc.sync.dma_start(out=t_q, in_=qw_bf[:, 128 * e:128 * (e + 1)],
                              transpose=True)
            qwT.append(t_q)
            t_o = wpool.tile([128, E], BF16, name=f"owT{e}")
            nc.sync.dma_start(out=t_o, in_=ow_bf[:, 128 * e:128 * (e + 1)],
                              transpose=True)
            owT.append(t_o)
        for e in range(ECT):
            t_v = wpool.tile([128, E], BF16, name=f"vwT{e}")
            nc.sync.dma_start(out=t_v, in_=vw_bf[:, 128 * e:128 * (e + 1)],
                              transpose=True)
            vwT.append(t_v)

        # pools reused across batches
        vpool = ctx.enter_context(tc.tile_pool(name="vpool", bufs=1))
        xpool = ctx.enter_context(tc.tile_pool(name="xpool", bufs=1))
        qpool = ctx.enter_context(tc.tile_pool(name="qpool", bufs=2))
        apool = ctx.enter_context(tc.tile_pool(name="apool", bufs=2))
        rpool = ctx.enter_context(tc.tile_pool(name="rpool", bufs=2))
        kvpool = ctx.enter_context(tc.tile_pool(name="kvpool", bufs=2))
        outpool = ctx.enter_context(tc.tile_pool(name="outpool", bufs=2))
        ps_qo = ctx.enter_context(tc.tile_pool(name="ps_qo", bufs=3, space="PSUM"))
        ps_d = ctx.enter_context(tc.tile_pool(name="ps_d", bufs=2, space="PSUM"))
        ps_mix = ctx.enter_context(tc.tile_pool(name="ps_mix", bufs=3,
                                                space="PSUM"))

        for b in range(b_core):
            # ---------------- v projection ----------------
            cross_f32 = vpool.tile([LC, EC], FP32, name="cross_f32")
            nc.sync.dma_start(out=cross_f32, in_=cross_d[b])
            cross_bf = vpool.tile([LC, EC], BF16, name="cross_bf")
            nc.vector.tensor_copy(cross_bf, cross_f32)
            crossT = []
            for e in range(ECT):
                psx = ps_mix.tile([128, LC], BF16, tag="ps_mix")
                nc.tensor.transpose(psx, cross_bf[:, 128 * e:128 * (e + 1)],
                                    identity[:LC, :LC])
                ct = vpool.tile([128, LC], BF16, name=f"crossT{e}")
                nc.vector.tensor_copy(ct, psx)
                crossT.append(ct)
            vT = []
            for p in range(FT):
                psv = ps_mix.tile([128, LC], FP32, tag="ps_mix")
                for e in range(ECT):
                    nc.tensor.matmul(psv, lhsT=vwT[e][:, 128 * p:128 * (p + 1)],
                                     rhs=crossT[e][:, :LC],
                                     start=(e == 0), stop=(e == ECT - 1))
                vt = vpool.tile([128, LC], BF16, name=f"vT{p}")
                nc.scalar.activation(vt, psv, ident, bias=vb_sb[:, p:p + 1],
                                     scale=1.0)
                vT.append(vt)
            vh = []
            for h in range(H):
                hi = h % 2
                psh = ps_mix.tile([LC, DH], BF16, tag="ps_mix")
                nc.tensor.transpose(psh, vT[h // 2][64 * hi:64 * (hi + 1), :LC],
                                    identity[64 * hi:64 * hi + DH,
                                             64 * hi:64 * hi + DH])
                vht = vpool.tile([LC, DH], BF16, name=f"vh{h}")
                nc.vector.tensor_copy(vht, psh)
                vh.append(vht)

            for blk in range(n_blocks):
                t0 = blk * t_block
                # ---------------- x^T for this block (2 halves) ----------------
                xT = []  # xT[e] = [128, t_block]
                for e in range(ET):
                    xt = xpool.tile([128, t_block], BF16, name=f"xT{e}")
                    xT.append(xt)
                for half in range(2):
                    hh = slice(half * t_half, (half + 1) * t_half)
                    for e in range(ET):
                        nc.sync.dma_start(
                            out=xT[e][:, hh],
                            in_=x_bf[b, t0 + half * t_half:
                                     t0 + (half + 1) * t_half,
                                     128 * e:128 * (e + 1)],
                            transpose=True)

                def emit_scores_exp(j, hi):
                    hs = slice(64 * hi, 64 * (hi + 1))
                    exp_t = apool.tile([LC, t_block], BF16, name="exp_t")
                    for c in range(t_block // T_CHUNK):
                        cs = slice(c * T_CHUNK, (c + 1) * T_CHUNK)
                        pss = ps_mix.tile([LC, T_CHUNK], FP32, tag="ps_mix")
                        nc.tensor.matmul(pss, lhsT=vT[j][hs, :LC],
                                         rhs=qT_t[hs, cs],
                                         start=True, stop=True)
                        nc.scalar.activation(exp_t[:, cs], pss, expf,
                                             scale=0.125)
                    return exp_t

                def emit_normalize(exp_t):
                    # denominator broadcast rows via ones-matmul, reciprocal
                    # on DVE, normalize multiply on GPSIMD
                    attn_t = apool.tile([LC, t_block], BF16, name="attn_t")
                    for c2 in range(t_block // 1024):
                        c2s = slice(c2 * 1024, (c2 + 1) * 1024)
                        recip = rpool.tile([LC, 1024], FP32, name="recip")
                        for q4 in range(2):
                            qs_l = slice(q4 * 512, (q4 + 1) * 512)
                            qs_g = slice(c2 * 1024 + q4 * 512,
                                         c2 * 1024 + (q4 + 1) * 512)
                            psd = ps_d.tile([LC, 512], FP32, tag="ps_d")
                            nc.tensor.matmul(psd, lhsT=ones77,
                                             rhs=exp_t[:, qs_g],
                                             start=True, stop=True)
                            nc.vector.reciprocal(recip[:, qs_l], psd)
                        nc.gpsimd.tensor_mul(attn_t[:, c2s],
                                             exp_t[:, c2s], recip)
                    return attn_t

                def emit_qkv(h, attn_t):
                    # qkv with buggy-reshape layout;
                    # attn_t free dim is t-local = 16*g_local + r
                    attn_r = attn_t[:].rearrange("p (g r) -> p r g", r=16)
                    kv_sb = []
                    for r2 in range(8):
                        psk = ps_mix.tile([128, g_block], FP32, tag="ps_mix")
                        nc.tensor.matmul(psk[0:64, :], lhsT=vh[h],
                                         rhs=attn_r[:, 2 * r2, :],
                                         start=True, stop=True)
                        nc.tensor.matmul(psk[64:128, :], lhsT=vh[h],
                                         rhs=attn_r[:, 2 * r2 + 1, :],
                                         start=True, stop=True,
                                         tile_position=(0, 64))
                        kv = kvpool.tile([128, g_block], BF16, name=f"kv{r2}")
                        nc.vector.tensor_copy(kv, psk)
                        kv_sb.append(kv)
                    return kv_sb

                def emit_oproj(b, h, blk, kv_sb):
                    for ti in range(g_block // 128):
                        trow = h * lt16 + blk * g_block + ti * 128
                        tis = slice(ti * 128, (ti + 1) * 128)
                        ou = outpool.tile([128, E], FP32, name="ou")
                        for fc in range(E // T_CHUNK):
                            fs = slice(fc * T_CHUNK, (fc + 1) * T_CHUNK)
                            pso = ps_qo.tile([128, T_CHUNK], FP32,
                                             tag="ps_qo")
                            for ep in range(ET):
                                nc.tensor.matmul(pso, lhsT=kv_sb[ep][:, tis],
                                                 rhs=owT[ep][:, fs],
                                                 start=(ep == 0), stop=False)
                            nc.tensor.matmul(pso, lhsT=ones1[:, :128],
                                             rhs=ob_sb[:, fs],
                                             start=False, stop=True)
                            nc.vector.tensor_copy(ou[:, fs], pso)
                        nc.sync.dma_start(out=out_d[b, trow:trow + 128, :],
                                          in_=ou)

                for j in range(FT):
                    # ---------------- q-proj for f-tile j ----------------
                    qT_t = qpool.tile([128, t_block], BF16, name="qT_t")
                    for c in range(t_block // T_CHUNK):
                        cs = slice(c * T_CHUNK, (c + 1) * T_CHUNK)
                        psq = ps_qo.tile([128, T_CHUNK], FP32, tag="ps_qo")
                        for e in range(ET):
                            nc.tensor.matmul(
                                psq,
                                lhsT=qwT[e][:, 128 * j:128 * (j + 1)],
                                rhs=xT[e][:, cs],
                                start=(e == 0), stop=(e == ET - 1))
                        nc.scalar.activation(qT_t[:, cs], psq, ident,
                                             bias=qb_sb[:, j:j + 1], scale=1.0)

                    # interleave the two heads so PE work (scores/denoms of
                    # one head) hides the recip/mul latency of the other
                    h0, h1 = 2 * j, 2 * j + 1
                    exp0 = emit_scores_exp(j, 0)
                    attn0 = emit_normalize(exp0)
                    exp1 = emit_scores_exp(j, 1)
                    kv0 = emit_qkv(h0, attn0)
                    attn1 = emit_normalize(exp1)
                    kv1 = emit_qkv(h1, attn1)
                    emit_oproj(b, h0, blk, kv0)
                    emit_oproj(b, h1, blk, kv1)


def build_program(b_core=B_CORE, l_tokens=L):
    nc = bacc.Bacc(None, target_bir_lowering=False, debug=False)
    x_d = nc.dram_tensor("x", [b_core, l_tokens, E], FP32, kind="ExternalInput")
    cross_d = nc.dram_tensor("cross", [b_core, LC, EC], FP32, kind="ExternalInput")
    qw_d = nc.dram_tensor("q_w", [E, E], FP32, kind="ExternalInput")
    qb_d = nc.dram_tensor("q_b", [E], FP32, kind="ExternalInput")
    vw_d = nc.dram_tensor("v_w", [E, EC], FP32, kind="ExternalInput")
    vb_d = nc.dram_tensor("v_b", [E], FP32, kind="ExternalInput")
    ow_d = nc.dram_tensor("o_w", [E, E], FP32, kind="ExternalInput")
    ob_d = nc.dram_tensor("o_b", [E], FP32, kind="ExternalInput")
    out_d = nc.dram_tensor("out", [b_core, l_tokens, E], FP32,
                           kind="ExternalOutput")
    with tile.TileContext(nc) as tc:
        emit(tc, out_d, x_d, cross_d, qw_d, qb_d, vw_d, vb_d, ow_d, ob_d,
             b_core=b_core, l_tokens=l_tokens)
    nc.finalize()
    return nc


_PROGRAM_CACHE = {}


def kernel(**inputs):
    from concourse import bass_utils

    x = np.ascontiguousarray(np.asarray(inputs["x"], dtype=np.float32))
    cross = np.ascontiguousarray(np.asarray(inputs["cross"], dtype=np.float32))
    weights = {
        k: np.ascontiguousarray(np.asarray(inputs[k], dtype=np.float32))
        for k in ("q_w", "q_b", "v_w", "v_b", "o_w", "o_b")
    }

    if "nc" not in _PROGRAM_CACHE:
        _PROGRAM_CACHE["nc"] = build_program()
    nc = _PROGRAM_CACHE["nc"]

    in_maps = []
    for i in range(N_CORES):
        m = {"x": x[B_CORE * i:B_CORE * (i + 1)],
             "cross": cross[B_CORE * i:B_CORE * (i + 1)]}
        m.update(weights)
        in_maps.append(m)

    res = bass_utils.run_bass_kernel_spmd(nc, in_maps,
                                          core_ids=list(range(N_CORES)))
    out = np.concatenate([r["out"] for r in res.results], axis=0)
    return out
